# revision 9
# baseline (speedup 1.0000x reference)
"""Trainium2 Bass kernel for nn_InterpLnr (ragged segment-wise linear resampling).

Contract: kernel(**inputs) takes the FULL unsharded inputs
  x: (16, 2176, 128) f32, scales: (1040,) f32, len_seq: (16,) int,
  len_seg_raw: (1040, 1) int
and returns the full (16, 2048, 128) f32 output.

Strategy (fully data-parallel, 2 output batches per core on 8 cores):
  The reference masks/compacts interpolated rows globally, then reshapes the
  compacted buffer to (16, L) and truncates/pads to 2048 columns. Each output
  row (b, t) is a 2-point linear interpolation of two adjacent rows of x at a
  data-dependent position. The host computes the tiny index/weight arrays
  (one int32 + two f32 per output row, exact IEEE f32 math identical to the
  reference); each NeuronCore does the heavy data movement: indirect-DMA
  gathers of 1KB row-pairs (2 MB per batch), a 3-instruction DVE
  interpolation, and a contiguous 1 MB store per batch.

  HW indirect-DMA semantics (probed): each dest PARTITION consumes exactly
  one index and reads its whole free extent contiguously from the source.
  So each gather uses a [128, 1] index column and a (128, 256) dest slice:
  partition p reads rows [idx[p], idx[p]+1] of x in one 1KB descriptor.
  Output row t = p*16 + k lives on partition p, pair-slot k (16 gathers
  per batch).
"""

import os
import sys

import numpy as np

for _p in ("/opt/trn_rl_repo", "/root/.axon_site/_ro/trn_rl_repo"):
    if os.path.isdir(_p) and _p not in sys.path:
        sys.path.append(_p)

import concourse.bacc as bacc
import concourse.mybir as mybir
import concourse.tile as tile
from concourse import bass_utils
from concourse.bass import IndirectOffsetOnAxis

MAX_LEN_SEQ = 2048
MAX_LEN_PAD = 2176
MIN_LEN_SEG = 32
S = 65
B = 16
D = 128
R = B * S
W = 256
T = MAX_LEN_PAD
NCORES = 8
BPC = B // NCORES          # output batches per core
CH = MAX_LEN_SEQ // 128    # 16 row-pair slots per partition per batch


def _precompute(scales, len_seq, len_seg_raw):
    """Per-output-row source index / interpolation weights, (16, 2048) each.

    Mirrors the reference's f32 arithmetic exactly (numpy = IEEE = XLA CPU).
    Invalid rows (t >= L) get index 0 with zero weights -> exact zeros.
    """
    sc = scales.astype(np.float32) + np.float32(0.5)
    len_seg = len_seg_raw.reshape(R).astype(np.int64) + MIN_LEN_SEG
    ls = len_seg.reshape(B, S)
    offset = np.concatenate(
        [np.zeros((B, 1), np.int64), np.cumsum(ls, axis=1)[:, :-1]], axis=1
    ).reshape(R)
    len_rp = np.repeat(len_seq.astype(np.int64), S)

    w = np.arange(W, dtype=np.float32)
    idx_scaled = w[None, :] / sc[:, None]
    idx_fl = np.floor(idx_scaled)
    lam = (idx_scaled - idx_fl).astype(np.float32)
    mask1 = idx_fl < (len_seg.astype(np.float32) - 1.0)[:, None]
    idx_org = idx_fl + offset.astype(np.float32)[:, None]
    mask2 = idx_org < (len_rp.astype(np.float32) - 1.0)[:, None]
    mask = mask1 & mask2

    cnt = mask.sum(axis=1).astype(np.int64)
    ends = np.cumsum(cnt)
    total = int(ends[-1])
    L = total // B

    src = np.zeros((B, MAX_LEN_SEQ), np.int32)
    a = np.zeros((B, MAX_LEN_SEQ), np.float32)
    c = np.zeros((B, MAX_LEN_SEQ), np.float32)
    nvalid = min(L, MAX_LEN_SEQ)
    t = np.arange(nvalid)
    for b in range(B):
        g = b * L + t
        r = np.searchsorted(ends, g, side="right")
        ww = (g - (ends[r] - cnt[r])).astype(np.int64)
        i_fl = idx_org[r, ww].astype(np.int32)
        src[b, :nvalid] = (r // S).astype(np.int32) * T + i_fl
        lamv = lam[r, ww]
        a[b, :nvalid] = np.float32(1.0) - lamv
        c[b, :nvalid] = lamv
    return src, a, c


def _build_nc():
    nc = bacc.Bacc("TRN2", target_bir_lowering=False)
    x = nc.dram_tensor("x", (B * T, D), mybir.dt.float32, kind="ExternalInput")
    idx = nc.dram_tensor("idx", (BPC, 128, CH), mybir.dt.int32, kind="ExternalInput")
    av = nc.dram_tensor("av", (BPC, 128, CH), mybir.dt.float32, kind="ExternalInput")
    cv = nc.dram_tensor("cv", (BPC, 128, CH), mybir.dt.float32, kind="ExternalInput")
    out = nc.dram_tensor(
        "out", (BPC * MAX_LEN_SEQ, D), mybir.dt.float32, kind="ExternalOutput"
    )
    # partition p of batch j holds output rows p*CH .. p*CH+CH-1 (8KB contig)
    out_v = out.ap().rearrange("(j p k) d -> j p (k d)", j=BPC, p=128, k=CH)

    with tile.TileContext(nc) as tc:
        with tc.tile_pool(name="pool", bufs=2) as pool:
            for j in range(BPC):
                idx_t = pool.tile([128, CH], mybir.dt.int32, tag="idx")
                av_t = pool.tile([128, CH], mybir.dt.float32, tag="av")
                cv_t = pool.tile([128, CH], mybir.dt.float32, tag="cv")
                nc.sync.dma_start(out=idx_t[:], in_=idx.ap()[j])
                nc.sync.dma_start(out=av_t[:], in_=av.ap()[j])
                nc.sync.dma_start(out=cv_t[:], in_=cv.ap()[j])

                # pair[p, k*256:(k+1)*256] = x rows [idx[p,k], idx[p,k]+1]:
                # one [128,1] index column per gather, 1KB per partition.
                pair = pool.tile([128, CH * 2 * D], mybir.dt.float32, tag="pair")
                for k in range(CH):
                    nc.gpsimd.indirect_dma_start(
                        out=pair[:, k * 2 * D : (k + 1) * 2 * D],
                        out_offset=None,
                        in_=x.ap(),
                        in_offset=IndirectOffsetOnAxis(
                            ap=idx_t[:, k : k + 1], axis=0
                        ),
                    )

                pv = pair[:].rearrange("p (k c) -> p k c", c=2 * D)
                left = pv[:, :, 0:D]
                right = pv[:, :, D : 2 * D]
                a_b = av_t[:].unsqueeze(2).broadcast_to([128, CH, D])
                c_b = cv_t[:].unsqueeze(2).broadcast_to([128, CH, D])

                res = pool.tile([128, CH * D], mybir.dt.float32, tag="res")
                tmp = pool.tile([128, CH * D], mybir.dt.float32, tag="tmp")
                res_v = res[:].rearrange("p (k d) -> p k d", d=D)
                tmp_v = tmp[:].rearrange("p (k d) -> p k d", d=D)
                nc.vector.tensor_mul(out=res_v, in0=left, in1=a_b)
                nc.vector.tensor_mul(out=tmp_v, in0=right, in1=c_b)
                nc.vector.tensor_add(out=res[:], in0=res[:], in1=tmp[:])

                nc.sync.dma_start(out=out_v[j], in_=res[:])
    nc.compile()
    return nc


_NC = None


def _get_nc():
    global _NC
    if _NC is None:
        _NC = _build_nc()
    return _NC


def make_in_maps(x, scales, len_seq, len_seg_raw):
    """Shard full inputs into per-core input maps."""
    xf = np.ascontiguousarray(x.astype(np.float32, copy=False).reshape(B * T, D))
    src, a, c = _precompute(scales, len_seq, len_seg_raw)
    in_maps = []
    for core in range(NCORES):
        bs = slice(core * BPC, (core + 1) * BPC)
        in_maps.append(
            {
                "x": xf,
                "idx": np.ascontiguousarray(src[bs].reshape(BPC, 128, CH)),
                "av": np.ascontiguousarray(a[bs].reshape(BPC, 128, CH)),
                "cv": np.ascontiguousarray(c[bs].reshape(BPC, 128, CH)),
            }
        )
    return in_maps


def kernel(**inputs):
    x = np.asarray(inputs["x"])
    scales = np.asarray(inputs["scales"], dtype=np.float32)
    len_seq = np.asarray(inputs["len_seq"])
    len_seg_raw = np.asarray(inputs["len_seg_raw"])

    in_maps = make_in_maps(x, scales, len_seq, len_seg_raw)
    res = bass_utils.run_bass_kernel_spmd(
        _get_nc(), in_maps, core_ids=list(range(NCORES))
    )
    out = np.concatenate(
        [res.results[core]["out"].reshape(BPC, MAX_LEN_SEQ, D) for core in range(NCORES)],
        axis=0,
    )
    return out.astype(np.float32, copy=False)
